# revision 42
# baseline (speedup 1.0000x reference)
"""Trainium2 Bass kernel for nn_BayesianBSpline1D.

reference:
    t = tanh(x); bases = quintic B-spline basis of t (knots linspace(-2,2,21))
    spline_out = mean_s(bases @ (coeff_mean + eps*std).T)  -> [N,1]
    kl = scalar from coeff_mean/coeff_log_var only

Mathematically spline_out[i] = f(tanh(x_i)) where f is the quintic spline with
collapsed coefficients c_eff = coeff_mean + mean(eps,0)*std (15 values).

Device work (memory-bound, pure elementwise): y = P(tanh(x)) where P is an
adaptive-degree (18-26) minimax-ish Chebyshev fit of f on [-1,1], evaluated
in factored (root-product) form for f32 stability (plain monomial Horner
loses ~3e-2 rel at deg 24; the factored chain is exact to fit error):
    P(t) = lead * (quartic leaf) * prod(cubic / quadratic factors)
Factor->instruction mapping (custom DVE ops registered at import):
    leaf    (q1)(q2) via u*(u+da*t+db), 8 stages, 4 params, 1 instr
    cubic   v*(t^3+at^2+bt+c) one instr per real root of the fit
    quad    v*(t^2+at+b)[*lead] one instr
    gquad   offloaded quad (t+B)^2+C: ACT Square(+Identity bias) + GPSIMD
            multiply -- balances DVE (the bottleneck) against idle ACT/Pool
tanh runs on ACT (measured 1e-7 vs f64). Last tile runs all-DVE so no
GPSIMD/ACT work trails the kernel. Per-core ~50us (TimelineSim).

Sharding: pure data-parallel over 8 NeuronCores; x split in 8 equal row
chunks, output concatenated. kl computed on host (O(15) work).
"""
import os
import sys

sys.path.insert(0, "/opt/trn_rl_repo")

import numpy as np

import concourse.bass as bass
import concourse.bacc as bacc
import concourse.mybir as mybir
from concourse.tile import TileContext
from concourse.bass_utils import run_bass_kernel_spmd
from concourse.dve_spec import Spec, Src0, Src1, C0, C1, C2, C3, sq, lower
import concourse.dve_ops as dvo
from concourse.dve_ops import _spill_c3_to_src1
from concourse.dve_uop import DveOpSpec

N = 4194304
N_CORES = 8
N_PER_CORE = N // N_CORES           # 524288
TILE_C = 1024
N_OFFLOAD = 3                        # quad factors moved to ACT+GPSIMD

N_BASIS, ORDER = 10, 5
H = 2.0 / N_BASIS
KNOTS = np.linspace(-1 - ORDER * H, 1 + ORDER * H, N_BASIS + 2 * ORDER + 1)
PRIOR_VAR = 25.0

TRACE = bool(int(os.environ.get("BASS_KERNEL_TRACE", "0")))
LAST_EXEC_NS = None  # set after each kernel() call when TRACE


# ---------------------------------------------------------------- custom ops
def _make_op(name, body, reference, spill_c3=False):
    if spill_c3:
        body = _spill_c3_to_src1(body)
    spec = Spec(body=body, reference=reference)
    shas = {}
    for ver in ("v3", "v4"):
        s = DveOpSpec(name=name, opcode=0, uops=lower(spec, ver=ver), rd1_en=True)
        shas[ver] = s.sha(ver)
    op = dvo.DveOp(name, spec, subdim=False, uops_sha=shas)
    dvo.OPS.append(op)
    dvo.CUSTOM_DVE_SPECS[name] = spec
    dvo._SUB_OPCODE_FOR_NAME[name] = dvo._CUSTOM_DVE_ROW_BASE + len(dvo.OPS) - 1
    assert dvo._SUB_OPCODE_FOR_NAME[name] < 0x20, "custom DVE row overflow"
    return op


if "LEAF2Q_ANT" not in dvo._SUB_OPCODE_FOR_NAME:
    # u = t^2 + s0 t + s1;  out = u * (u + imm2 t + in1)
    # == (t^2+s0 t+s1)(t^2 + (s0+imm2) t + (s1+in1));  in1: [P,1] broadcast
    _u = sq(Src0) + C0 * Src0 + C1
    LEAF2Q_ANT = _make_op(
        "LEAF2Q_ANT",
        _u * (_u + C2 * Src0 + Src1),
        lambda in0, in1, s0, s1, imm2: (
            (in0 * in0 + s0 * in0 + s1)
            * (in0 * in0 + (s0 + imm2) * in0 + (s1 + in1))
        ).astype(np.float32),
    )
    # out = v * (t^2 + s0 t + s1);  in0 = v, in1 = t
    CHAINQ_ANT = _make_op(
        "CHAINQ_ANT",
        Src0 * (sq(Src1) + C0 * Src1 + C1),
        lambda in0, in1, s0, s1, imm2: (
            in0 * (in1 * in1 + s0 * in1 + s1)
        ).astype(np.float32),
    )
    # out = v * (t^2 + s0 t + s1) * imm2
    CHAINQS_ANT = _make_op(
        "CHAINQS_ANT",
        Src0 * (sq(Src1) + C0 * Src1 + C1) * C2,
        lambda in0, in1, s0, s1, imm2: (
            in0 * (in1 * in1 + s0 * in1 + s1) * imm2
        ).astype(np.float32),
    )
    # out = v * (t^3 + s0 t^2 + s1 t + imm2)   (general monic cubic factor)
    CUBIC_ANT = _make_op(
        "CUBIC_ANT",
        Src0 * (((Src1 + C0) * Src1 + C1) * Src1 + C2),
        lambda in0, in1, s0, s1, imm2: (
            in0 * (((in1 + s0) * in1 + s1) * in1 + imm2)
        ).astype(np.float32),
    )
else:  # re-import in same process
    LEAF2Q_ANT = next(o for o in dvo.OPS if o.name == "LEAF2Q_ANT")
    CHAINQ_ANT = next(o for o in dvo.OPS if o.name == "CHAINQ_ANT")
    CHAINQS_ANT = next(o for o in dvo.OPS if o.name == "CHAINQS_ANT")
    CUBIC_ANT = next(o for o in dvo.OPS if o.name == "CUBIC_ANT")


# ---------------------------------------------------------------- host math
def _b_splines(x, knots, k):
    left, right = knots[:-1], knots[1:]
    xe = x[:, None]
    basis = ((xe >= left) & (xe < right)).astype(np.float64)
    basis[:, -1] = np.where(x == knots[-1], 1.0, basis[:, -1])
    n = len(knots)
    for d in range(1, k + 1):
        m = n - d - 1
        d1 = knots[d:d + m] - knots[:m]
        d2 = knots[d + 1:d + 1 + m] - knots[1:1 + m]
        t1 = np.where(d1 != 0, (xe - knots[:m]) / np.where(d1 != 0, d1, 1), 0) * basis[:, :m]
        t2 = np.where(d2 != 0, (knots[d + 1:d + 1 + m] - xe) / np.where(d2 != 0, d2, 1), 0) * basis[:, 1:1 + m]
        basis = t1 + t2
    return basis


def _fit_cheb(tg, y, deg):
    w = np.ones_like(tg)
    best = None
    for _ in range(8):
        cch = np.polynomial.chebyshev.chebfit(tg, y, deg, w=w)
        r = np.abs(np.polynomial.chebyshev.chebval(tg, cch) - y)
        if best is None or r.max() < best[0]:
            best = (r.max(), cch)
        w = (1e-9 + r) ** 0.3
        w /= w.mean()
    return best


def _plan_factors(cch):
    """Build the op plan from a chebyshev fit.

    Returns a list of op descriptors:
      ("leaf", a1, b1, da, db)      (t^2+a1 t+b1)(t^2+(a1+da)t+(b1+db))
      ("quad", a, b, s)             v * (t^2+a t+b) * s     (s==None -> CHAINQ)
      ("cubic", c2, c1, c0)         v * (t^3+c2 t^2+c1 t+c0)
      ("scale", s)                  v * s  (stock tensor_scalar fallback)
    """
    mono = np.polynomial.chebyshev.cheb2poly(cch)
    while len(mono) > 3 and abs(mono[-1]) < 1e-300:
        mono = mono[:-1]
    p = np.polynomial.Polynomial(mono)
    roots = p.roots()
    lead = float(mono[-1])
    reals = sorted(r.real for r in roots if abs(r.imag) <= 1e-9)
    pairs = [(-2 * r.real, float(abs(r) ** 2)) for r in roots if r.imag > 1e-9]
    pairs.sort(key=lambda ab: -ab[0] / 2)

    cubics = []  # (real, pair) or (r1, r2, r3)
    # greedily: one real + the conj-pair with closest root real part
    while reals and pairs:
        r = reals.pop(0)
        j = min(range(len(pairs)), key=lambda i: abs(-pairs[i][0] / 2 - r))
        cubics.append(("rp", r, pairs.pop(j)))
    while len(reals) >= 3:
        cubics.append(("rrr", reals.pop(0), reals.pop(-1), reals.pop(len(reals) // 2)))
    quads = list(pairs)
    while len(reals) >= 2:
        r1, r2 = reals.pop(0), reals.pop(-1)
        quads.append((-(r1 + r2), r1 * r2))
    if reals:  # single leftover real: steal from a cubic
        r0 = reals.pop()
        kind = cubics.pop()
        if kind[0] == "rp":
            _, r, pr = kind
            quads.append(pr)
            quads.append((-(r + r0), r * r0))
        else:
            _, r1, r2, r3 = kind
            quads.append((-(r1 + r2), r1 * r2))
            quads.append((-(r3 + r0), r3 * r0))
    while len(quads) < 2 and cubics:  # need 2 quads for the leaf
        kind = cubics.pop()
        if kind[0] == "rp":
            _, r, pr = kind
            quads.append(pr)
            reals.append(r)
        else:
            _, r1, r2, r3 = kind
            quads.append((-(r1 + r2), r1 * r2))
            reals.append(r3)
    if len(reals) == 2:
        quads.append((-(reals[0] + reals[1]), reals[0] * reals[1]))
        reals = []
    assert not reals or len(quads) >= 2

    def cub_coeffs(kind):
        if kind[0] == "rp":
            _, r, (a, b) = kind
            return (a - r, b - r * a, -r * b)
        _, r1, r2, r3 = kind
        return (-(r1 + r2 + r3), r1 * r2 + r1 * r3 + r2 * r3, -r1 * r2 * r3)

    if len(quads) >= 2:
        # leaf takes the two quads with most-similar b (small |db| keeps f32 tidy)
        quads.sort(key=lambda ab: ab[1])
        q1, q2 = quads.pop(0), quads.pop(0)
        plan = [("leaf", q1[0], q1[1], q2[0] - q1[0], q2[1] - q1[1])]
    else:
        # degenerate: no pairs at all -> all real roots; build leaf from 2 real-quads
        plan = [("leaf", 0.0, 0.0, 0.0, 0.0)]  # (t^2)(t^2): placeholder, adjust lead
        raise RuntimeError("degenerate root structure; unhandled")

    # offload up to N_OFFLOAD quads to ACT(square)+GPSIMD:
    #   (t^2+at+b) = (t+a/2)^2 + (b-a^2/4)
    # mode "actadd": ACT does square AND +C (2 ACT, 1 Pool mult)
    # mode "pooladd": ACT square only (1 ACT, 2 Pool ops)
    goff = []
    qlist = [("quad", a, b, None) for (a, b) in quads]
    while len(goff) < N_OFFLOAD and len(qlist) > 1:  # keep >=1 DVE quad for lead
        _, a, b, _ = qlist.pop()
        mode = "actadd" if len(goff) < 2 else "pooladd"
        goff.append(("gquad", a / 2.0, b - a * a / 4.0, mode))
    # chain: interleave quads and cubics, balanced ordering
    chain = qlist + [("cubic",) + cub_coeffs(k) for k in cubics]
    chain.sort(key=lambda op: op[1])
    chain = chain[0::2] + chain[1::2][::-1]
    # attach lead to the first quad op; else add a stock scale op
    for i, op in enumerate(chain):
        if op[0] == "quad":
            chain[i] = ("quad", op[1], op[2], lead)
            break
    else:
        chain.append(("scale", lead))
    # gpsimd quads go at the END of the chain: each tile's DVE portion
    # finishes first and the DVE engine moves on to the next tile while
    # GPSIMD completes this one.
    return plan + chain + goff


def _simulate_f32(t, plan):
    """Exact f32 simulation of the device op chain (excluding tanh)."""
    f = np.float32
    v = None
    for op in plan:
        if op[0] == "leaf":
            _, a, b, da, db = op
            u = (t * t + f(a) * t + f(b)).astype(f)
            v = (u * (u + f(da) * t + f(db))).astype(f)
        elif op[0] == "quad":
            _, a, b, s = op
            q = (t * t + f(a) * t + f(b)).astype(f)
            v = (v * q).astype(f)
            if s is not None:
                v = (v * f(s)).astype(f)
        elif op[0] == "cubic":
            _, c2, c1, c0 = op
            q = ((((t + f(c2)) * t + f(c1)) * t) + f(c0)).astype(f)
            v = (v * q).astype(f)
        elif op[0] == "gquad":
            beta, cc = op[1], op[2]
            w = ((t + f(beta)) * (t + f(beta))).astype(f)
            v = ((w + f(cc)) * v).astype(f)
        elif op[0] == "scale":
            v = (v * f(op[1])).astype(f)
    return v


def _plan_factors_simple(cch):
    """Fallback: all factors as plain quadratic chain ops (no cubics/offload)."""
    mono = np.polynomial.chebyshev.cheb2poly(cch)
    while len(mono) > 3 and abs(mono[-1]) < 1e-300:
        mono = mono[:-1]
    p = np.polynomial.Polynomial(mono)
    roots = p.roots()
    lead = float(mono[-1])
    reals = sorted(r.real for r in roots if abs(r.imag) <= 1e-9)
    quads = [(-2 * r.real, float(abs(r) ** 2)) for r in roots if r.imag > 1e-9]
    while len(reals) >= 2:
        r1, r2 = reals.pop(0), reals.pop(-1)
        quads.append((-(r1 + r2), r1 * r2))
    quads.sort(key=lambda ab: ab[0])
    quads = quads[0::2] + quads[1::2][::-1]
    q1, q2 = quads[0], quads[1]
    plan = [("leaf", q1[0], q1[1], q2[0] - q1[0], q2[1] - q1[1])]
    plan.append(("quad", quads[2][0], quads[2][1], lead))
    plan += [("quad", a, b, None) for (a, b) in quads[3:]]
    return plan


def _host_prepare(c_eff):
    tg = np.linspace(-1, 1, 4001)
    y = _b_splines(tg, KNOTS, ORDER) @ c_eff
    scale = float(np.abs(y).max())
    if scale < 1e-30:
        return None  # output identically ~0
    # adaptive degree: smallest fit hitting target, capped
    best = None
    for deg in (18, 20, 22, 24, 26):
        efit, cch = _fit_cheb(tg, y, deg)
        best = (efit, cch)
        if efit / scale <= 4e-4:
            break
    efit, cch = best
    tg32 = tg.astype(np.float32)
    plan = None
    try:
        plan = _plan_factors(cch)
        v32 = _simulate_f32(tg32, plan).astype(np.float64)
        e32 = float(np.abs(v32 - y).max())
        if not np.isfinite(e32) or e32 > 3 * efit + 1e-6 * scale:
            plan = None
    except Exception:
        plan = None
    if plan is None:  # fallback: plain quad chain
        plan = _plan_factors_simple(cch)
        v32 = _simulate_f32(tg32, plan).astype(np.float64)
        e32 = float(np.abs(v32 - y).max())
    return plan, efit / scale, e32 / scale


# ---------------------------------------------------------------- bass build
def _build_nc(plan, tile_c=TILE_C, tile_sizes=None):
    f32 = mybir.dt.float32
    A = mybir.AluOpType
    total_c = N_PER_CORE // 128  # 4096 columns of 128 partitions
    if tile_sizes is None:
        tile_sizes = [tile_c] * (total_c // tile_c)
    assert sum(tile_sizes) == total_c
    tile_offs = np.cumsum([0] + tile_sizes[:-1]).tolist()
    n_tiles = len(tile_sizes)
    tile_c = max(tile_sizes)  # const tiles sized to the widest tile
    nc = bacc.Bacc(
        "TRN2", target_bir_lowering=False, debug=False, num_devices=N_CORES
    )
    xv = nc.dram_tensor("x", [N_PER_CORE, 1], f32, kind="ExternalInput").ap()
    yv = nc.dram_tensor("y", [N_PER_CORE, 1], f32, kind="ExternalOutput").ap()
    xv = xv.rearrange("(p c) o -> p (c o)", p=128)
    yv = yv.rearrange("(p c) o -> p (c o)", p=128)

    assert plan[0][0] == "leaf"
    leaf = plan[0]
    chain = plan[1:]
    # last tile runs everything on DVE so no GPSIMD work trails the kernel
    chain_full = [
        ("quad", 2 * op[1], op[1] * op[1] + op[2], None) if op[0] == "gquad" else op
        for op in chain
    ]

    with TileContext(nc) as tc:
        with (
            tc.tile_pool(name="cst", bufs=1) as cst,
            tc.tile_pool(name="io", bufs=3) as io,
            tc.tile_pool(name="wk", bufs=3) as wk,
            tc.tile_pool(name="res", bufs=3) as res,
        ):
            b2 = cst.tile([128, tile_c], f32, tag="b2")
            nc.gpsimd.memset(b2[:], float(leaf[4]))
            gq_bias = {}
            gq_c = {}
            for op in chain:
                if op[0] == "gquad":
                    bt = cst.tile([128, 1], f32, tag=f"gqb{len(gq_bias)}")
                    nc.gpsimd.memset(bt[:], float(op[1]))
                    gq_bias[id(op)] = bt
                    if op[3] == "actadd":
                        ct = cst.tile([128, 1], f32, tag=f"gqc{len(gq_c)}")
                    else:
                        ct = cst.tile([128, tile_c], f32, tag=f"gqc{len(gq_c)}")
                    nc.gpsimd.memset(ct[:], float(op[2]))
                    gq_c[id(op)] = ct
            for i in range(n_tiles):
                off, w_c = tile_offs[i], tile_sizes[i]
                # small tiles (ramp/tail) run everything on DVE; offload only
                # on full-width tiles so no GPSIMD/ACT work trails the kernel
                tile_chain = chain if (w_c == tile_c and i < n_tiles - 1) else chain_full
                xt = io.tile([128, w_c], f32, tag="x")
                nc.sync.dma_start(out=xt[:], in_=xv[:, off:off + w_c])
                tt = wk.tile([128, w_c], f32, tag="t")
                nc.scalar.activation(tt[:], xt[:], mybir.ActivationFunctionType.Tanh)
                cur = wk.tile([128, w_c], f32, tag="va")
                nc.vector._custom_dve(
                    LEAF2Q_ANT, out=cur[:], in0=tt[:], in1=b2[:, :w_c],
                    s0=float(leaf[1]), s1=float(leaf[2]), imm2=float(leaf[3]),
                )
                for ci, op in enumerate(tile_chain):
                    last = ci == len(tile_chain) - 1
                    pool = res if last else wk
                    tag = "y" if last else ("vb" if ci % 2 == 0 else "vc")
                    nxt = pool.tile([128, w_c], f32, tag=tag)
                    if op[0] == "quad" and op[3] is None:
                        nc.vector._custom_dve(
                            CHAINQ_ANT, out=nxt[:], in0=cur[:], in1=tt[:],
                            s0=float(op[1]), s1=float(op[2]),
                        )
                    elif op[0] == "quad":
                        nc.vector._custom_dve(
                            CHAINQS_ANT, out=nxt[:], in0=cur[:], in1=tt[:],
                            s0=float(op[1]), s1=float(op[2]), imm2=float(op[3]),
                        )
                    elif op[0] == "cubic":
                        nc.vector._custom_dve(
                            CUBIC_ANT, out=nxt[:], in0=cur[:], in1=tt[:],
                            s0=float(op[1]), s1=float(op[2]), imm2=float(op[3]),
                        )
                    elif op[0] == "gquad":
                        w = wk.tile([128, w_c], f32, tag="w")
                        nc.scalar.activation(
                            w[:], tt[:], mybir.ActivationFunctionType.Square,
                            bias=gq_bias[id(op)][:],
                        )
                        w2 = wk.tile([128, w_c], f32, tag="w2")
                        if op[3] == "actadd":
                            nc.scalar.activation(
                                w2[:], w[:],
                                mybir.ActivationFunctionType.Identity,
                                bias=gq_c[id(op)][:],
                            )
                        else:
                            nc.gpsimd.tensor_tensor(
                                w2[:], w[:], gq_c[id(op)][:, :w_c], A.add
                            )
                        nc.gpsimd.tensor_tensor(nxt[:], w2[:], cur[:], A.mult)
                    elif op[0] == "scale":
                        nc.vector.tensor_scalar(
                            nxt[:], cur[:], float(op[1]), None, A.mult
                        )
                    cur = nxt
                nc.sync.dma_start(out=yv[:, off:off + w_c], in_=cur[:])
    nc.compile()
    return nc


# ---------------------------------------------------------------- entry
def kernel(**inputs):
    global LAST_EXEC_NS
    x = np.ascontiguousarray(np.asarray(inputs["x"], dtype=np.float32))
    coeff_mean = np.asarray(inputs["coeff_mean"], dtype=np.float64)
    coeff_log_var = np.asarray(inputs["coeff_log_var"], dtype=np.float64)
    eps = np.asarray(inputs["eps"], dtype=np.float64)

    std = np.exp(0.5 * coeff_log_var)
    c_eff = coeff_mean + eps.mean(0) * std
    var = np.exp(coeff_log_var)
    kl = np.float32(
        0.5 * np.sum(
            (var + coeff_mean ** 2) / PRIOR_VAR - 1.0 - np.log(var / PRIOR_VAR)
        )
    )

    prep = _host_prepare(c_eff)
    if prep is None:
        return np.zeros((N, 1), np.float32), kl
    plan, efit_rel, e32_rel = prep

    nc = _build_nc(plan)
    chunks = x.reshape(N_CORES, N_PER_CORE, 1)
    in_maps = [{"x": chunks[i]} for i in range(N_CORES)]
    r = None
    for attempt in range(3):
        try:
            r = run_bass_kernel_spmd(nc, in_maps, list(range(N_CORES)), trace=False)
            break
        except Exception:
            # transient axon/NRT hiccups have been observed; retry
            if attempt == 2:
                raise
            import time as _time
            _time.sleep(15)
    LAST_EXEC_NS = r.exec_time_ns
    y = np.concatenate([m["y"] for m in r.results], axis=0)
    return y, kl


if __name__ == "__main__":
    rng = np.random.default_rng(0)
    x = rng.standard_normal((N, 1)).astype(np.float32)
    cm = (rng.standard_normal(15) * 0.05).astype(np.float32)
    clv = np.full(15, -5.0, np.float32)
    eps = rng.standard_normal((8, 15)).astype(np.float32)
    y, kl = kernel(x=x, coeff_mean=cm, coeff_log_var=clv, eps=eps)
    print("y", y.shape, y.dtype, "kl", kl)


# revision 46
# speedup vs baseline: 3.3093x; 3.3093x over previous
"""Trainium2 Bass kernel for nn_BayesianBSpline1D.

reference:
    t = tanh(x); bases = quintic B-spline basis of t (knots linspace(-2,2,21))
    spline_out = mean_s(bases @ (coeff_mean + eps*std).T)  -> [N,1]
    kl = scalar from coeff_mean/coeff_log_var only

Mathematically spline_out[i] = f(tanh(x_i)) where f is the quintic spline with
collapsed coefficients c_eff = coeff_mean + mean(eps,0)*std (15 values).

Device work (memory-bound, pure elementwise): y = P(tanh(x)) where P is an
adaptive-degree (18-26) minimax-ish Chebyshev fit of f on [-1,1], evaluated
in factored (root-product) form for f32 stability (plain monomial Horner
loses ~3e-2 rel at deg 24; the factored chain is exact to fit error):
    P(t) = lead * (quartic leaf) * prod(cubic / quadratic factors)
Factor->instruction mapping (custom DVE ops registered at import):
    leaf    (q1)(q2) via u*(u+da*t+db), 8 stages, 4 params, 1 instr
    cubic   v*(t^3+at^2+bt+c) one instr per real root of the fit
    quad    v*(t^2+at+b)[*lead] one instr
    gquad   offloaded quad (t+B)^2+C: ACT Square(+Identity bias) + GPSIMD
            multiply -- balances DVE (the bottleneck) against idle ACT/Pool
tanh runs on ACT (measured 1e-7 vs f64). Last tile runs all-DVE so no
GPSIMD/ACT work trails the kernel. Per-core ~50us (TimelineSim).

Sharding: pure data-parallel over 8 NeuronCores; x split in 8 equal row
chunks, output concatenated. kl computed on host (O(15) work).
"""
import os
import sys

sys.path.insert(0, "/opt/trn_rl_repo")

import numpy as np

import concourse.bass as bass
import concourse.bacc as bacc
import concourse.mybir as mybir
from concourse.tile import TileContext
from concourse.bass_utils import run_bass_kernel_spmd
from concourse.dve_spec import Spec, Src0, Src1, C0, C1, C2, C3, sq, lower
import concourse.dve_ops as dvo
from concourse.dve_ops import _spill_c3_to_src1
from concourse.dve_uop import DveOpSpec

N = 4194304
N_CORES = 8
N_PER_CORE = N // N_CORES           # 524288
TILE_C = 1024
N_OFFLOAD = 3                        # quad factors moved to ACT+GPSIMD

N_BASIS, ORDER = 10, 5
H = 2.0 / N_BASIS
KNOTS = np.linspace(-1 - ORDER * H, 1 + ORDER * H, N_BASIS + 2 * ORDER + 1)
PRIOR_VAR = 25.0

TRACE = bool(int(os.environ.get("BASS_KERNEL_TRACE", "0")))
LAST_EXEC_NS = None  # set after each kernel() call when TRACE


# ---------------------------------------------------------------- custom ops
def _make_op(name, body, reference, spill_c3=False):
    if spill_c3:
        body = _spill_c3_to_src1(body)
    spec = Spec(body=body, reference=reference)
    shas = {}
    for ver in ("v3", "v4"):
        s = DveOpSpec(name=name, opcode=0, uops=lower(spec, ver=ver), rd1_en=True)
        shas[ver] = s.sha(ver)
    op = dvo.DveOp(name, spec, subdim=False, uops_sha=shas)
    dvo.OPS.append(op)
    dvo.CUSTOM_DVE_SPECS[name] = spec
    dvo._SUB_OPCODE_FOR_NAME[name] = dvo._CUSTOM_DVE_ROW_BASE + len(dvo.OPS) - 1
    assert dvo._SUB_OPCODE_FOR_NAME[name] < 0x20, "custom DVE row overflow"
    return op


if "LEAF2Q_ANT" not in dvo._SUB_OPCODE_FOR_NAME:
    # u = t^2 + s0 t + s1;  out = u * (u + imm2 t + in1)
    # == (t^2+s0 t+s1)(t^2 + (s0+imm2) t + (s1+in1));  in1: [P,1] broadcast
    _u = sq(Src0) + C0 * Src0 + C1
    LEAF2Q_ANT = _make_op(
        "LEAF2Q_ANT",
        _u * (_u + C2 * Src0 + Src1),
        lambda in0, in1, s0, s1, imm2: (
            (in0 * in0 + s0 * in0 + s1)
            * (in0 * in0 + (s0 + imm2) * in0 + (s1 + in1))
        ).astype(np.float32),
    )
    # out = v * (t^2 + s0 t + s1);  in0 = v, in1 = t
    CHAINQ_ANT = _make_op(
        "CHAINQ_ANT",
        Src0 * (sq(Src1) + C0 * Src1 + C1),
        lambda in0, in1, s0, s1, imm2: (
            in0 * (in1 * in1 + s0 * in1 + s1)
        ).astype(np.float32),
    )
    # out = v * (t^2 + s0 t + s1) * imm2
    CHAINQS_ANT = _make_op(
        "CHAINQS_ANT",
        Src0 * (sq(Src1) + C0 * Src1 + C1) * C2,
        lambda in0, in1, s0, s1, imm2: (
            in0 * (in1 * in1 + s0 * in1 + s1) * imm2
        ).astype(np.float32),
    )
    # out = v * (t^3 + s0 t^2 + s1 t + imm2)   (general monic cubic factor)
    CUBIC_ANT = _make_op(
        "CUBIC_ANT",
        Src0 * (((Src1 + C0) * Src1 + C1) * Src1 + C2),
        lambda in0, in1, s0, s1, imm2: (
            in0 * (((in1 + s0) * in1 + s1) * in1 + imm2)
        ).astype(np.float32),
    )
else:  # re-import in same process
    LEAF2Q_ANT = next(o for o in dvo.OPS if o.name == "LEAF2Q_ANT")
    CHAINQ_ANT = next(o for o in dvo.OPS if o.name == "CHAINQ_ANT")
    CHAINQS_ANT = next(o for o in dvo.OPS if o.name == "CHAINQS_ANT")
    CUBIC_ANT = next(o for o in dvo.OPS if o.name == "CUBIC_ANT")


# ---------------------------------------------------------------- host math
def _b_splines(x, knots, k):
    left, right = knots[:-1], knots[1:]
    xe = x[:, None]
    basis = ((xe >= left) & (xe < right)).astype(np.float64)
    basis[:, -1] = np.where(x == knots[-1], 1.0, basis[:, -1])
    n = len(knots)
    for d in range(1, k + 1):
        m = n - d - 1
        d1 = knots[d:d + m] - knots[:m]
        d2 = knots[d + 1:d + 1 + m] - knots[1:1 + m]
        t1 = np.where(d1 != 0, (xe - knots[:m]) / np.where(d1 != 0, d1, 1), 0) * basis[:, :m]
        t2 = np.where(d2 != 0, (knots[d + 1:d + 1 + m] - xe) / np.where(d2 != 0, d2, 1), 0) * basis[:, 1:1 + m]
        basis = t1 + t2
    return basis


def _fit_cheb(tg, y, deg):
    w = np.ones_like(tg)
    best = None
    for _ in range(8):
        cch = np.polynomial.chebyshev.chebfit(tg, y, deg, w=w)
        r = np.abs(np.polynomial.chebyshev.chebval(tg, cch) - y)
        if best is None or r.max() < best[0]:
            best = (r.max(), cch)
        w = (1e-9 + r) ** 0.3
        w /= w.mean()
    return best


def _plan_factors(cch):
    """Build the op plan from a chebyshev fit.

    Returns a list of op descriptors:
      ("leaf", a1, b1, da, db)      (t^2+a1 t+b1)(t^2+(a1+da)t+(b1+db))
      ("quad", a, b, s)             v * (t^2+a t+b) * s     (s==None -> CHAINQ)
      ("cubic", c2, c1, c0)         v * (t^3+c2 t^2+c1 t+c0)
      ("scale", s)                  v * s  (stock tensor_scalar fallback)
    """
    mono = np.polynomial.chebyshev.cheb2poly(cch)
    while len(mono) > 3 and abs(mono[-1]) < 1e-300:
        mono = mono[:-1]
    p = np.polynomial.Polynomial(mono)
    roots = p.roots()
    lead = float(mono[-1])
    reals = sorted(r.real for r in roots if abs(r.imag) <= 1e-9)
    pairs = [(-2 * r.real, float(abs(r) ** 2)) for r in roots if r.imag > 1e-9]
    pairs.sort(key=lambda ab: -ab[0] / 2)

    cubics = []  # (real, pair) or (r1, r2, r3)
    # greedily: one real + the conj-pair with closest root real part
    while reals and pairs:
        r = reals.pop(0)
        j = min(range(len(pairs)), key=lambda i: abs(-pairs[i][0] / 2 - r))
        cubics.append(("rp", r, pairs.pop(j)))
    while len(reals) >= 3:
        cubics.append(("rrr", reals.pop(0), reals.pop(-1), reals.pop(len(reals) // 2)))
    quads = list(pairs)
    while len(reals) >= 2:
        r1, r2 = reals.pop(0), reals.pop(-1)
        quads.append((-(r1 + r2), r1 * r2))
    if reals:  # single leftover real: steal from a cubic
        r0 = reals.pop()
        kind = cubics.pop()
        if kind[0] == "rp":
            _, r, pr = kind
            quads.append(pr)
            quads.append((-(r + r0), r * r0))
        else:
            _, r1, r2, r3 = kind
            quads.append((-(r1 + r2), r1 * r2))
            quads.append((-(r3 + r0), r3 * r0))
    while len(quads) < 2 and cubics:  # need 2 quads for the leaf
        kind = cubics.pop()
        if kind[0] == "rp":
            _, r, pr = kind
            quads.append(pr)
            reals.append(r)
        else:
            _, r1, r2, r3 = kind
            quads.append((-(r1 + r2), r1 * r2))
            reals.append(r3)
    if len(reals) == 2:
        quads.append((-(reals[0] + reals[1]), reals[0] * reals[1]))
        reals = []
    assert not reals or len(quads) >= 2

    def cub_coeffs(kind):
        if kind[0] == "rp":
            _, r, (a, b) = kind
            return (a - r, b - r * a, -r * b)
        _, r1, r2, r3 = kind
        return (-(r1 + r2 + r3), r1 * r2 + r1 * r3 + r2 * r3, -r1 * r2 * r3)

    if len(quads) >= 2:
        # leaf takes the two quads with most-similar b (small |db| keeps f32 tidy)
        quads.sort(key=lambda ab: ab[1])
        q1, q2 = quads.pop(0), quads.pop(0)
        plan = [("leaf", q1[0], q1[1], q2[0] - q1[0], q2[1] - q1[1])]
    else:
        # degenerate: no pairs at all -> all real roots; build leaf from 2 real-quads
        plan = [("leaf", 0.0, 0.0, 0.0, 0.0)]  # (t^2)(t^2): placeholder, adjust lead
        raise RuntimeError("degenerate root structure; unhandled")

    # offload up to N_OFFLOAD quads to ACT(square)+GPSIMD:
    #   (t^2+at+b) = (t+a/2)^2 + (b-a^2/4)
    # mode "actadd": ACT does square AND +C (2 ACT, 1 Pool mult)
    # mode "pooladd": ACT square only (1 ACT, 2 Pool ops)
    goff = []
    qlist = [("quad", a, b, None) for (a, b) in quads]
    while len(goff) < N_OFFLOAD and len(qlist) > 1:  # keep >=1 DVE quad for lead
        _, a, b, _ = qlist.pop()
        mode = "actadd" if len(goff) < 2 else "pooladd"
        goff.append(("gquad", a / 2.0, b - a * a / 4.0, mode))
    # chain: interleave quads and cubics, balanced ordering
    chain = qlist + [("cubic",) + cub_coeffs(k) for k in cubics]
    chain.sort(key=lambda op: op[1])
    chain = chain[0::2] + chain[1::2][::-1]
    # attach lead to the first quad op; else add a stock scale op
    for i, op in enumerate(chain):
        if op[0] == "quad":
            chain[i] = ("quad", op[1], op[2], lead)
            break
    else:
        chain.append(("scale", lead))
    # gpsimd quads go at the END of the chain: each tile's DVE portion
    # finishes first and the DVE engine moves on to the next tile while
    # GPSIMD completes this one.
    return plan + chain + goff


def _simulate_f32(t, plan):
    """Exact f32 simulation of the device op chain (excluding tanh)."""
    f = np.float32
    v = None
    for op in plan:
        if op[0] == "leaf":
            _, a, b, da, db = op
            u = (t * t + f(a) * t + f(b)).astype(f)
            v = (u * (u + f(da) * t + f(db))).astype(f)
        elif op[0] == "quad":
            _, a, b, s = op
            q = (t * t + f(a) * t + f(b)).astype(f)
            v = (v * q).astype(f)
            if s is not None:
                v = (v * f(s)).astype(f)
        elif op[0] == "cubic":
            _, c2, c1, c0 = op
            q = ((((t + f(c2)) * t + f(c1)) * t) + f(c0)).astype(f)
            v = (v * q).astype(f)
        elif op[0] == "gquad":
            beta, cc = op[1], op[2]
            w = ((t + f(beta)) * (t + f(beta))).astype(f)
            v = ((w + f(cc)) * v).astype(f)
        elif op[0] == "scale":
            v = (v * f(op[1])).astype(f)
    return v


def _plan_factors_simple(cch):
    """Fallback: all factors as plain quadratic chain ops (no cubics/offload)."""
    mono = np.polynomial.chebyshev.cheb2poly(cch)
    while len(mono) > 3 and abs(mono[-1]) < 1e-300:
        mono = mono[:-1]
    p = np.polynomial.Polynomial(mono)
    roots = p.roots()
    lead = float(mono[-1])
    reals = sorted(r.real for r in roots if abs(r.imag) <= 1e-9)
    quads = [(-2 * r.real, float(abs(r) ** 2)) for r in roots if r.imag > 1e-9]
    while len(reals) >= 2:
        r1, r2 = reals.pop(0), reals.pop(-1)
        quads.append((-(r1 + r2), r1 * r2))
    quads.sort(key=lambda ab: ab[0])
    quads = quads[0::2] + quads[1::2][::-1]
    q1, q2 = quads[0], quads[1]
    plan = [("leaf", q1[0], q1[1], q2[0] - q1[0], q2[1] - q1[1])]
    plan.append(("quad", quads[2][0], quads[2][1], lead))
    plan += [("quad", a, b, None) for (a, b) in quads[3:]]
    return plan


def _host_prepare(c_eff):
    tg = np.linspace(-1, 1, 4001)
    y = _b_splines(tg, KNOTS, ORDER) @ c_eff
    scale = float(np.abs(y).max())
    if scale < 1e-30:
        return None  # output identically ~0
    # adaptive degree: smallest fit hitting target, capped
    best = None
    for deg in (18, 20, 22, 24, 26):
        efit, cch = _fit_cheb(tg, y, deg)
        best = (efit, cch)
        if efit / scale <= 4e-4:
            break
    efit, cch = best
    tg32 = tg.astype(np.float32)
    plan = None
    try:
        plan = _plan_factors(cch)
        v32 = _simulate_f32(tg32, plan).astype(np.float64)
        e32 = float(np.abs(v32 - y).max())
        if not np.isfinite(e32) or e32 > 3 * efit + 1e-6 * scale:
            plan = None
    except Exception:
        plan = None
    if plan is None:  # fallback: plain quad chain
        plan = _plan_factors_simple(cch)
        v32 = _simulate_f32(tg32, plan).astype(np.float64)
        e32 = float(np.abs(v32 - y).max())
    return plan, efit / scale, e32 / scale


# ---------------------------------------------------------------- bass build
def _build_nc(plan, tile_c=TILE_C, tile_sizes=None):
    f32 = mybir.dt.float32
    A = mybir.AluOpType
    total_c = N_PER_CORE // 128  # 4096 columns of 128 partitions
    if tile_sizes is None:
        tile_sizes = [tile_c] * (total_c // tile_c)
    assert sum(tile_sizes) == total_c
    tile_offs = np.cumsum([0] + tile_sizes[:-1]).tolist()
    n_tiles = len(tile_sizes)
    tile_c = max(tile_sizes)  # const tiles sized to the widest tile
    nc = bacc.Bacc(
        "TRN2", target_bir_lowering=False, debug=False, num_devices=N_CORES
    )
    xv = nc.dram_tensor("x", [N_PER_CORE, 1], f32, kind="ExternalInput").ap()
    yv = nc.dram_tensor("y", [N_PER_CORE, 1], f32, kind="ExternalOutput").ap()
    xv = xv.rearrange("(p c) o -> p (c o)", p=128)
    yv = yv.rearrange("(p c) o -> p (c o)", p=128)

    assert plan[0][0] == "leaf"
    leaf = plan[0]
    chain = plan[1:]
    # last tile runs everything on DVE so no GPSIMD work trails the kernel
    chain_full = [
        ("quad", 2 * op[1], op[1] * op[1] + op[2], None) if op[0] == "gquad" else op
        for op in chain
    ]

    with TileContext(nc) as tc:
        with (
            tc.tile_pool(name="cst", bufs=1) as cst,
            tc.tile_pool(name="io", bufs=3) as io,
            tc.tile_pool(name="wk", bufs=3) as wk,
            tc.tile_pool(name="res", bufs=3) as res,
        ):
            b2 = cst.tile([128, tile_c], f32, tag="b2")
            nc.gpsimd.memset(b2[:], float(leaf[4]))
            gq_bias = {}
            gq_c = {}
            for op in chain:
                if op[0] == "gquad":
                    bt = cst.tile([128, 1], f32, tag=f"gqb{len(gq_bias)}")
                    nc.gpsimd.memset(bt[:], float(op[1]))
                    gq_bias[id(op)] = bt
                    if op[3] == "actadd":
                        ct = cst.tile([128, 1], f32, tag=f"gqc{len(gq_c)}")
                    else:
                        ct = cst.tile([128, tile_c], f32, tag=f"gqc{len(gq_c)}")
                    nc.gpsimd.memset(ct[:], float(op[2]))
                    gq_c[id(op)] = ct
            for i in range(n_tiles):
                off, w_c = tile_offs[i], tile_sizes[i]
                # small tiles (ramp/tail) run everything on DVE; offload only
                # on full-width tiles so no GPSIMD/ACT work trails the kernel
                tile_chain = chain if (w_c == tile_c and i < n_tiles - 1) else chain_full
                xt = io.tile([128, w_c], f32, tag="x")
                nc.sync.dma_start(out=xt[:], in_=xv[:, off:off + w_c])
                tt = wk.tile([128, w_c], f32, tag="t")
                nc.scalar.activation(tt[:], xt[:], mybir.ActivationFunctionType.Tanh)
                cur = wk.tile([128, w_c], f32, tag="va")
                nc.vector._custom_dve(
                    LEAF2Q_ANT, out=cur[:], in0=tt[:], in1=b2[:, :w_c],
                    s0=float(leaf[1]), s1=float(leaf[2]), imm2=float(leaf[3]),
                )
                for ci, op in enumerate(tile_chain):
                    last = ci == len(tile_chain) - 1
                    pool = res if last else wk
                    tag = "y" if last else ("vb" if ci % 2 == 0 else "vc")
                    nxt = pool.tile([128, w_c], f32, tag=tag)
                    if op[0] == "quad" and op[3] is None:
                        nc.vector._custom_dve(
                            CHAINQ_ANT, out=nxt[:], in0=cur[:], in1=tt[:],
                            s0=float(op[1]), s1=float(op[2]),
                        )
                    elif op[0] == "quad":
                        nc.vector._custom_dve(
                            CHAINQS_ANT, out=nxt[:], in0=cur[:], in1=tt[:],
                            s0=float(op[1]), s1=float(op[2]), imm2=float(op[3]),
                        )
                    elif op[0] == "cubic":
                        nc.vector._custom_dve(
                            CUBIC_ANT, out=nxt[:], in0=cur[:], in1=tt[:],
                            s0=float(op[1]), s1=float(op[2]), imm2=float(op[3]),
                        )
                    elif op[0] == "gquad":
                        w = wk.tile([128, w_c], f32, tag="w")
                        nc.scalar.activation(
                            w[:], tt[:], mybir.ActivationFunctionType.Square,
                            bias=gq_bias[id(op)][:],
                        )
                        w2 = wk.tile([128, w_c], f32, tag="w2")
                        if op[3] == "actadd":
                            nc.scalar.activation(
                                w2[:], w[:],
                                mybir.ActivationFunctionType.Identity,
                                bias=gq_c[id(op)][:],
                            )
                        else:
                            nc.gpsimd.tensor_tensor(
                                w2[:], w[:], gq_c[id(op)][:, :w_c], A.add
                            )
                        nc.gpsimd.tensor_tensor(nxt[:], w2[:], cur[:], A.mult)
                    elif op[0] == "scale":
                        nc.vector.tensor_scalar(
                            nxt[:], cur[:], float(op[1]), None, A.mult
                        )
                    cur = nxt
                nc.sync.dma_start(out=yv[:, off:off + w_c], in_=cur[:])
    nc.compile()
    return nc


# ------------------------------------------------- ACT-table (v2) fast path
V2_SCALE = 0.8
V2_BIAS = 4.9
V2_CLAMP = 5.9


def _g_exact(u, c_eff):
    t = np.tanh((np.asarray(u, np.float64) - V2_BIAS) / V2_SCALE)
    return _b_splines(np.clip(t, -1.0, 1.0), KNOTS, ORDER) @ c_eff


def _build_act_dir(c_eff, tmpdir):
    """Copy the compiler's pwp act tables; rewrite derivative_erf's bucket
    coefficients to g(u) = f(tanh((u-bias)/scale)). Grid/ctrl/meta kept
    verbatim. Returns (act_info_path, content_hash)."""
    import shutil
    import hashlib
    import neuronxcc

    src = os.path.join(os.path.dirname(neuronxcc.__file__), "pwp", "pwp_bin_trainium")
    dst = os.path.join(tmpdir, "act_custom")
    shutil.copytree(src, dst)
    import json

    prof = json.load(open(os.path.join(dst, "erf_derivative.json")))
    bkt_path = os.path.join(dst, prof["bkt_bin"])
    bkt = np.fromfile(bkt_path, dtype=np.float32).reshape(-1, 8).copy()
    starts = prof["func_to_bkt_start_idx"]
    s0 = starts["derivative_erf"]
    s1 = min(v for v in starts.values() if v > s0)
    mids = bkt[s0:s1, 4].astype(np.float64)
    order = np.argsort(mids)
    sorted_mids = mids[order]
    gaps = np.diff(sorted_mids)
    half = {}
    for j, oi in enumerate(order):
        lo = gaps[j - 1] if j > 0 else gaps[0]
        hi = gaps[j] if j < len(gaps) else gaps[-1]
        half[oi] = max(lo, hi) / 2.0
    for i in range(s1 - s0):
        m = mids[i]
        h = half[i]
        uu = np.linspace(m - h, m + h, 13)
        gg = _g_exact(uu, c_eff)
        cf = np.polynomial.polynomial.polyfit(uu - m, gg, 3)
        bkt[s0 + i, 0:4] = np.asarray(cf, np.float32)
    bkt.tofile(bkt_path)
    hsh = hashlib.sha256(bkt.tobytes()).digest()
    hval = int.from_bytes(hsh[:4], "little")
    return os.path.join(dst, "act_info.json"), hval


def _build_nc_v2(clamp_hi, tile_sizes=None, clamp=True, bias_eps=0.0):
    f32 = mybir.dt.float32
    A = mybir.AluOpType
    total_c = N_PER_CORE // 128
    if tile_sizes is None:
        tile_sizes = [1024] * (total_c // 1024)
    assert sum(tile_sizes) == total_c
    tile_offs = np.cumsum([0] + tile_sizes[:-1]).tolist()
    nc = bacc.Bacc(
        "TRN2", target_bir_lowering=False, debug=False, num_devices=N_CORES
    )
    xv = nc.dram_tensor("x", [N_PER_CORE, 1], f32, kind="ExternalInput").ap()
    yv = nc.dram_tensor("y", [N_PER_CORE, 1], f32, kind="ExternalOutput").ap()
    xv = xv.rearrange("(p c) o -> p (c o)", p=128)
    yv = yv.rearrange("(p c) o -> p (c o)", p=128)
    with TileContext(nc) as tc:
        with (
            tc.tile_pool(name="cst", bufs=1) as cst,
            tc.tile_pool(name="io", bufs=3) as io,
            tc.tile_pool(name="wk", bufs=3) as wk,
            tc.tile_pool(name="res", bufs=4) as res,
        ):
            bias = cst.tile([128, 1], f32, tag="bias")
            nc.gpsimd.memset(bias[:], float(V2_BIAS + bias_eps))
            # dummy activation so LoadActFuncSet happens during the first DMA
            dum = cst.tile([128, 1], f32, tag="dum")
            nc.gpsimd.memset(dum[:], 1.0)
            dout = cst.tile([128, 1], f32, tag="dout")
            nc.scalar.activation(
                dout[:], dum[:], mybir.ActivationFunctionType.Derivative_Erf,
                bias=bias[:], scale=float(V2_SCALE),
            )
            for i in range(len(tile_sizes)):
                off, w_c = tile_offs[i], tile_sizes[i]
                xt = io.tile([128, w_c], f32, tag="x")
                nc.sync.dma_start(out=xt[:], in_=xv[:, off:off + w_c])
                if clamp:
                    ct = wk.tile([128, w_c], f32, tag="c")
                    nc.vector.tensor_scalar(
                        ct[:], xt[:], float(clamp_hi), float(-V2_CLAMP),
                        A.min, A.max,
                    )
                else:
                    ct = xt
                yt = res.tile([128, w_c], f32, tag="y")
                nc.scalar.activation(
                    yt[:], ct[:],
                    mybir.ActivationFunctionType.Derivative_Erf,
                    bias=bias[:], scale=float(V2_SCALE),
                )
                nc.sync.dma_start(out=yv[:, off:off + w_c], in_=yt[:])
    nc.compile()
    return nc


def _run_spmd(nc, in_maps):
    for attempt in range(3):
        try:
            return run_bass_kernel_spmd(
                nc, in_maps, list(range(N_CORES)), trace=False
            )
        except Exception:
            if attempt == 2:
                raise
            import time as _time
            _time.sleep(15)


def _kernel_v2(x, c_eff):
    """One-ACT-op path via rewritten derivative_erf tables. Returns y or
    raises; caller self-checks and falls back to the DVE chain."""
    import tempfile

    td = tempfile.mkdtemp(prefix="actlut_")
    act_info, hval = _build_act_dir(c_eff, td)
    os.environ["BASS_ACT_ROOT_JSON_PATH"] = act_info
    try:
        # fold the table hash into the clamp immediate so the HLO hash (and
        # hence the NEFF cache key) changes whenever the table content does
        clamp_hi = V2_CLAMP + (hval % 65536) * 1e-11
        nc = _build_nc_v2(clamp_hi)
        chunks = x.reshape(N_CORES, N_PER_CORE, 1)
        in_maps = [{"x": chunks[i]} for i in range(N_CORES)]
        r = _run_spmd(nc, in_maps)
        return np.concatenate([m["y"] for m in r.results], axis=0)
    finally:
        os.environ.pop("BASS_ACT_ROOT_JSON_PATH", None)


def _self_check(y, x, c_eff, scale):
    rng = np.random.default_rng(0)
    idx = rng.integers(0, N, 4096)
    t = np.tanh(x[idx, 0].astype(np.float64))
    exp = _b_splines(t, KNOTS, ORDER) @ c_eff
    err = np.abs(y[idx, 0].astype(np.float64) - exp).max()
    return err <= 2e-3 * scale


# ---------------------------------------------------------------- entry
def kernel(**inputs):
    global LAST_EXEC_NS
    x = np.ascontiguousarray(np.asarray(inputs["x"], dtype=np.float32))
    coeff_mean = np.asarray(inputs["coeff_mean"], dtype=np.float64)
    coeff_log_var = np.asarray(inputs["coeff_log_var"], dtype=np.float64)
    eps = np.asarray(inputs["eps"], dtype=np.float64)

    std = np.exp(0.5 * coeff_log_var)
    c_eff = coeff_mean + eps.mean(0) * std
    var = np.exp(coeff_log_var)
    kl = np.float32(
        0.5 * np.sum(
            (var + coeff_mean ** 2) / PRIOR_VAR - 1.0 - np.log(var / PRIOR_VAR)
        )
    )

    tg = np.linspace(-1, 1, 4001)
    yg = _b_splines(tg, KNOTS, ORDER) @ c_eff
    scale = float(np.abs(yg).max())
    if scale < 1e-30:
        return np.zeros((N, 1), np.float32), kl

    # fast path: single rewritten-ACT-table op; self-checked on real outputs
    if os.environ.get("BASS_KERNEL_V1", "0") != "1":
        try:
            y = _kernel_v2(x, c_eff)
            if _self_check(y, x, c_eff, scale):
                return y, kl
        except Exception:
            pass

    # fallback: factored custom-DVE polynomial chain (proven path)
    prep = _host_prepare(c_eff)
    plan, efit_rel, e32_rel = prep
    nc = _build_nc(plan)
    chunks = x.reshape(N_CORES, N_PER_CORE, 1)
    in_maps = [{"x": chunks[i]} for i in range(N_CORES)]
    r = _run_spmd(nc, in_maps)
    LAST_EXEC_NS = r.exec_time_ns
    y = np.concatenate([m["y"] for m in r.results], axis=0)
    return y, kl


if __name__ == "__main__":
    rng = np.random.default_rng(0)
    x = rng.standard_normal((N, 1)).astype(np.float32)
    cm = (rng.standard_normal(15) * 0.05).astype(np.float32)
    clv = np.full(15, -5.0, np.float32)
    eps = rng.standard_normal((8, 15)).astype(np.float32)
    y, kl = kernel(x=x, coeff_mean=cm, coeff_log_var=clv, eps=eps)
    print("y", y.shape, y.dtype, "kl", kl)
